# revision 20
# baseline (speedup 1.0000x reference)
"""DirectAU loss kernel for Trainium2, SPMD over 8 NeuronCores (v5).

Math (see reference):
  user_e = user_table[user_id]; pos_e = item_table[pos_id]   (B=8192, D=64)
  align  = mean_i ||un_i - pn_i||^2 = 2 - (2/B) sum_i <un_i, pn_i>
  unif(x)= log( (sum_{i<j} exp(-4 + 4 <xn_i, xn_j>)) / npairs )
  out    = align + 0.5*(unif(user_e) + unif(pos_e))

Strategy (v5, evolved from v4 at 130.7us):
  - Same chunk-pair coverage as v4: cores 0-3 own the user-table Gram,
    4-7 the pos one; each core owns 2 adjacent 1024-row chunks {a1,a1+1}
    (a1=2j) and multiplies them against chunks a1..a1+5, with the
    distance-4 blocks split in complementary halves between core pairs
    via the half-swapped gather order of slots 4/5.
  - Gathers use gpsimd.dma_gather (994ns + 0.34ns/row) instead of 48
    per-band indirect DMAs (994ns EACH): batch rows are SORTED by table
    id so each 1024-row chunk spans a ~12.5K id range that fits int16
    indices against a per-core 32K-row table window (the Gram is
    permutation invariant, so any chunking works). 4 issues (~5us on
    GpSimd) replace ~56 (~65us) - the v4 pipeline was gather-issue
    paced end to end.
  - The alignment term needs batch-paired rows of BOTH tables on one
    core, which sorting scrambles; it is 0.01% of the FLOPs and is
    folded into the host-side finalization (which already applies the
    closed-form log / diagonal corrections) in float64.
  - Normalization square/reduce/rsqrt run on the otherwise-idle Pool
    engine; DVE does only the fused multiply+fp8-cast and the
    PSUM->SBUF transpose copies (Pool has no PSUM port).
  - Exp drains are the wall (ACT: 0.833ns/col + ~0.5us/instr overhead
    = ~73us for all 66560 cols/core). A tunable fraction of the 2048-
    wide PSUM drain tiles is instead evaluated as a Schraudolph
    bitcast-exp on DVE (tensor_scalar i32 affine, calibrated to
    +2e-4 mean bias on the Gram distribution) with the f32-bitcast
    reduce on Pool, splitting the exp wall across three engines.
  - fp8-e4m3 DoubleRow Gram matmuls as v4 (PE-transposed [32,2,512]
    k-tile layout); matmuls emitted lhs-major in long uninterrupted
    streams so the PE p-state ramps instead of idling at 0.65GHz.
"""

import math

import numpy as np

import concourse.bacc as bacc
import concourse.bass as bass
import concourse.mybir as mybir
import concourse.tile as tile
from concourse import bass_utils
from concourse.masks import make_identity
from concourse.tile_rust import add_dep_helper

B = 8192
DIM = 64
NROWS = 100000
NCORES = 8
CHUNK = 1024
NSLOT = 6            # gathered main chunks per core (slots 0..5)
BANDS = NSLOT * 8    # 48 gather bands of 128 rows
NGRP = NSLOT * 2     # transpose groups of 4 bands (512 rows)
NPAIR = 3            # dma_gather windows (2 chunks each)
WINDOW = 32768       # rows per window (int16 index reach)
F32 = mybir.dt.float32
F8 = mybir.dt.float8e4
I16 = mybir.dt.int16
I32 = mybir.dt.int32

PSW = 2048           # PSUM work tile width (fp32)
ACC_W = 48
SELF_COL0 = 0        # self-tile accum columns (host removes diag double count)
OFF_COL0 = 4         # off-diagonal accum columns

# Schraudolph fast-exp for exp(4s-4): i32 = s*A + B_ , bitcast f32.
# B_ calibrated (C=-480000) for ~2e-4 mean bias over <xn_i,xn_j> ~ N(0,1/64).
A_SCH = float(np.float32(4.0 * (2.0 ** 23) / math.log(2.0)))
B_SCH = float(np.float32(127 * 2 ** 23 - 4.0 * (2.0 ** 23) / math.log(2.0) - 480000.0))

# off-drain k -> use DVE bitcast-exp instead of ACT exp (Pool cannot touch
# PSUM or run fp32 ALU ops, so the exp wall splits across ACT/DVE only)
def _dve_drain(k):
    return k >= 3 and (k % 4) == 1

_COLS = {"self": 0, "off": 0}  # filled at build; read by _finalize


def _emit_rsqrt(eng, pool, x_ap, out_ap, n, tag, order):
    """out = 1/sqrt(x) (bit-hack seed + 2 Newton steps) on engine `eng`."""
    MAGIC = 0x5F3759DF
    op = mybir.AluOpType
    ti = pool.tile([128, n], I32, tag=f"{tag}_ti", name=f"{tag}_ti")
    order(eng.tensor_scalar(
        out=ti[:], in0=x_ap.bitcast(I32), scalar1=1, scalar2=None,
        op0=op.logical_shift_right,
    ))
    yi = pool.tile([128, n], I32, tag=f"{tag}_yi", name=f"{tag}_yi")
    order(eng.tensor_scalar(
        out=yi[:], in0=ti[:], scalar1=-1, scalar2=None, op0=op.bitwise_xor
    ))
    order(eng.tensor_scalar(
        out=yi[:], in0=yi[:], scalar1=MAGIC + 1, scalar2=None, op0=op.add
    ))
    xh = pool.tile([128, n], F32, tag=f"{tag}_xh", name=f"{tag}_xh")
    order(eng.tensor_scalar(
        out=xh[:], in0=x_ap, scalar1=-0.5, scalar2=None, op0=op.mult
    ))
    cur = yi[:].bitcast(F32)
    for it in range(2):
        t2 = pool.tile([128, n], F32, tag=f"{tag}_t2", name=f"{tag}_t2")
        order(eng.tensor_tensor(out=t2[:], in0=cur, in1=cur, op=op.mult))
        order(eng.tensor_tensor(out=t2[:], in0=t2[:], in1=xh[:], op=op.mult))
        order(eng.tensor_scalar(
            out=t2[:], in0=t2[:], scalar1=1.5, scalar2=None, op0=op.add
        ))
        if it == 1:
            dst_ap = out_ap
        else:
            yt = pool.tile([128, n], F32, tag=f"{tag}_y", name=f"{tag}_y{it}")
            dst_ap = yt[:]
        order(eng.tensor_tensor(out=dst_ap, in0=cur, in1=t2[:], op=op.mult))
        cur = dst_ap


def _body(tc, wind, gidx, acc):
    nc = tc.nc
    op = mybir.AluOpType
    DR = mybir.MatmulPerfMode.DoubleRow
    with (
        tc.tile_pool(name="persist", bufs=1) as P,
        tc.tile_pool(name="work", bufs=2) as W,
        tc.tile_pool(name="ps", bufs=2, space="PSUM") as PS,
    ):
        idx_sb = P.tile([128, NPAIR * 128], I16, tag="idx")

        accw = P.tile([128, ACC_W], F32, tag="accw")
        bias_o = P.tile([128, 1], F32, tag="bias_o")
        ident = P.tile([128, 128], F32, tag="ident")
        ident8 = P.tile([128, 128], F8, tag="ident8")

        # gathered rows, [128, band, DIM]: row i of slot c -> partition i%128,
        # band c*8 + i//128 (dma_gather's native layout)
        gath = P.tile([128, BANDS * DIM], F32, tag="gath")
        gath8 = P.tile([128, BANDS * DIM], F8, tag="gath8")
        # fp8 transposed layout: group g (4 bands = 512 rows) occupies cols
        # [g*1024, (g+1)*1024): [32 partitions, k-half h in {0,1}, 512 rows]
        xnT8 = P.tile([32, NGRP * 1024], F8, tag="xnT8")
        nsq = P.tile([128, BANDS], F32, tag="nsq")
        rinv = P.tile([128, BANDS], F32, tag="rinv")

        # queue-order pinning (the scheduler cost model mis-predicts gather
        # and PE readiness; pin each in-order engine to emission order)
        last = {"pe": None, "act": None, "dve": None, "pool": None}

        def _mk(key):
            def f(inst):
                if last[key] is not None:
                    add_dep_helper(inst.ins, last[key].ins, sync=False,
                                   reason=f"{key} order")
                last[key] = inst
                return inst
            return f

        pe_o, act_o, dve_o, pool_o = _mk("pe"), _mk("act"), _mk("dve"), _mk("pool")

        def gather(slot0, nslots):
            """one dma_gather for slots [slot0, slot0+nslots) out of window
            slot0//2 (idx data is window-relative)."""
            p = slot0 // 2
            n = nslots * CHUNK
            pool_o(nc.gpsimd.dma_gather(
                out_ap=gath[:, slot0 * 8 * DIM : (slot0 + nslots) * 8 * DIM]
                .rearrange("q (c d) -> q c d", d=DIM),
                in_ap=wind[p * WINDOW : (p + 1) * WINDOW, :],
                idxs_ap=idx_sb[:, slot0 * 64 : slot0 * 64 + n // 16],
                num_idxs=n,
                num_idxs_reg=n,
                elem_size=DIM,
            ))

        def setup_consts():
            make_identity(nc, ident[:])
            dve_o(nc.vector.tensor_copy(out=ident8[:], in_=ident[:]))
            warm = P.tile([128, 1], F32, tag="warm")
            act_o(nc.scalar.activation(
                out=warm[:], in_=bias_o[:],
                func=mybir.ActivationFunctionType.Exp,
            ))

        def norm_pool(c0, c1):
            """square + reduce + rsqrt on DVE for bands [c0, c1) (Pool's ALU
            is integer-only and has no PSUM/fp32 path)."""
            nb = c1 - c0
            g3 = gath[:, c0 * DIM : c1 * DIM].rearrange("p (c d) -> p c d", d=DIM)
            sq = W.tile([128, 16 * DIM], F32, tag="sq", name=f"sq{c0}")
            dve_o(nc.vector.tensor_tensor(
                out=sq[:, 0 : nb * DIM], in0=g3, in1=g3, op=op.mult))
            dve_o(nc.vector.tensor_reduce(
                out=nsq[:, c0:c1],
                in_=sq[:, 0 : nb * DIM].rearrange("p (c d) -> p c d", d=DIM),
                axis=mybir.AxisListType.X,
                op=op.add,
            ))
            _emit_rsqrt(nc.vector, W, nsq[:, c0:c1], rinv[:, c0:c1], nb,
                        f"rs{c0}", dve_o)

        def norm_mul(c0, c1):
            """fused normalize-multiply + fp8 cast on DVE."""
            nb = c1 - c0
            g3 = gath[:, c0 * DIM : c1 * DIM].rearrange("p (c d) -> p c d", d=DIM)
            r3 = (
                rinv[:, c0:c1]
                .rearrange("p (c o) -> p c o", o=1)
                .to_broadcast([128, nb, DIM])
            )
            g83 = gath8[:, c0 * DIM : c1 * DIM].rearrange("p (c d) -> p c d", d=DIM)
            dve_o(nc.vector.tensor_tensor(out=g83, in0=g3, in1=r3, op=op.mult))

        def transpose_group(g):
            """8 fp8 transposes (4 bands x 2 halves) -> [32,1024] PSUM, then
            DVE-copy the packed group into xnT8 (estep-2 PSUM constraint)."""
            pt8 = PS.tile([32, 2048], F8, tag="ps", name=f"tp{g}")
            for bi in range(4):
                c = g * 4 + bi
                for h in range(2):
                    s = 2 * (h * 512 + bi * 128)
                    pe_o(nc.tensor.transpose(
                        out=pt8[0:32, s : s + 256 : 2],
                        in_=gath8[:, c * DIM + h * 32 : c * DIM + (h + 1) * 32],
                        identity=ident8[:],
                    ))
            dve_o(nc.vector.tensor_copy(
                out=xnT8[:, g * 1024 : (g + 1) * 1024], in_=pt8[0:32, 0:2048:2]
            ))

        def rhs_ap(g, co, w):
            return xnT8[:, g * 1024 : (g + 1) * 1024].rearrange(
                "p (h c) -> p h c", h=2
            )[:, :, co : co + w]

        def lhs_ap(q, rt):
            return rhs_ap(q * 2 + rt // 4, (rt % 4) * 128, 128)

        # ---- rolling drain emitter: 2048-wide tiles, segments by kind ----
        st = {"tile": None, "fill": 0, "segs": [], "n": 0}
        cols = {"self": SELF_COL0, "off": OFF_COL0}
        offk = [0]
        dvek = [0]

        def mm_piece(q, rt, g, co, w, kind="off"):
            lhs = lhs_ap(q, rt)
            while w > 0:
                if st["tile"] is None:
                    st["tile"] = PS.tile(
                        [128, PSW], F32, tag="ps", name=f"mm{st['n']}"
                    )
                    st["n"] += 1
                take = min(w, PSW - st["fill"], 512 - st["fill"] % 512)
                pe_o(nc.tensor.matmul(
                    out=st["tile"][:, st["fill"] : st["fill"] + take],
                    lhsT=lhs,
                    rhs=rhs_ap(g, co, take),
                    start=True,
                    stop=True,
                    perf_mode=DR,
                ))
                if st["segs"] and st["segs"][-1][2] == kind \
                        and st["segs"][-1][1] == st["fill"]:
                    st["segs"][-1] = (st["segs"][-1][0], st["fill"] + take, kind)
                else:
                    st["segs"].append((st["fill"], st["fill"] + take, kind))
                st["fill"] += take
                co += take
                w -= take
                if st["fill"] == PSW:
                    flush()

        def drain_act(ap_in, col):
            act_o(nc.scalar.activation(
                out=ap_in,
                in_=ap_in,
                func=mybir.ActivationFunctionType.Exp,
                bias=bias_o[:],
                scale=4.0,
                accum_out=accw[:, col : col + 1],
            ))

        deferred_tr = []

        def drain_dve(ap_in, w, col):
            # convert PSUM->SBUF immediately (frees the PSUM slot for PE so
            # ACT never starves on the 2-slot rotation); the reduce of the
            # staged tile is deferred into DVE slack / the tail.
            k = dvek[0]
            dvek[0] += 1
            cv = P.tile([128, PSW], I32, tag=f"cv{k}", name=f"cv{k}")
            dve_o(nc.vector.tensor_scalar(
                out=cv[:, 0:w], in0=ap_in, scalar1=A_SCH, scalar2=B_SCH,
                op0=op.mult, op1=op.add,
            ))
            deferred_tr.append((cv, w, col))

        def emit_deferred_tr():
            for cv, w, col in deferred_tr:
                dve_o(nc.vector.tensor_reduce(
                    out=accw[:, col : col + 1],
                    in_=cv[:, 0:w].bitcast(F32),
                    axis=mybir.AxisListType.X,
                    op=op.add,
                ))
            deferred_tr.clear()

        def flush():
            if st["fill"]:
                t = st["tile"]
                for (lo, hi, kind) in st["segs"]:
                    if kind == "self":
                        col = cols["self"]
                        cols["self"] += 1
                        drain_act(t[:, lo:hi], col)
                    else:
                        k = offk[0]
                        offk[0] += 1
                        col = cols["off"]
                        cols["off"] += 1
                        if _dve_drain(k):
                            drain_dve(t[:, lo:hi], hi - lo, col)
                        else:
                            drain_act(t[:, lo:hi], col)
            st["tile"] = None
            st["fill"] = 0
            st["segs"] = []

        def self_pass(q):
            for rt in range(8):
                mm_piece(q, rt, *(_lhs_loc(q, rt) + (128,)), kind="self")

        def _lhs_loc(q, rt):
            return (q * 2 + rt // 4, (rt % 4) * 128)

        def up_pass(q):
            # strict upper triangle of diag chunk q at 128-tile granularity
            for rt in range(8):
                s = (rt + 1) * 128
                for lo, hi in ((s, 512), (max(s, 512), 1024)):
                    if hi > lo:
                        mm_piece(q, rt, q * 2 + lo // 512, lo % 512, hi - lo)

        # ---- emission ----
        # NOTE: one dma_gather per 1024-row chunk. 2048-idx gathers emit 129
        # descriptors, one over the 128-deep SWDGE ring -> device lockup.
        # Gather descgen measures ~8.4ns/row of Pool-engine time (the cost
        # model's 0.34ns/desc is wrong for the gather ucode), so the ~52us
        # gather stream paces the kernel: idx load goes through the same
        # Pool SWDGE queue so gathers start at ~3us, and ALL downstream work
        # is emitted in slot-availability order to drain continuously.
        nc.gpsimd.memset(bias_o[:], -4.0)
        nc.gpsimd.memset(accw[:], 0.0)
        pool_o(nc.gpsimd.dma_start(out=idx_sb[:], in_=gidx))
        gather(0, 1)
        gather(1, 1)
        setup_consts()
        gather(2, 1)
        gather(3, 1)
        gather(4, 1)
        gather(5, 1)

        def norm_tp(slot):
            norm_pool(slot * 8, (slot + 1) * 8)
            norm_mul(slot * 8, (slot + 1) * 8)
            transpose_group(2 * slot)
            transpose_group(2 * slot + 1)

        # slot 0: q0 self + upper
        norm_tp(0)
        self_pass(0)
        up_pass(0)
        # slot 1: sibling block + q1 self + upper
        norm_tp(1)
        for rt in range(8):
            mm_piece(0, rt, 2, 0, 512)
            mm_piece(0, rt, 3, 0, 512)
        self_pass(1)
        up_pass(1)
        # slots 2,3: q0/q1 x slot
        for s in (2, 3):
            norm_tp(s)
            for q in (0, 1):
                for rt in range(8):
                    mm_piece(q, rt, 2 * s, 0, 512)
                    mm_piece(q, rt, 2 * s + 1, 0, 512)
        # slot 4: q0 distance-4 half (half-swapped) + q1 full
        norm_tp(4)
        for rt in range(8):
            mm_piece(0, rt, 8 if rt < 4 else 9, 0, 512)
        for rt in range(8):
            mm_piece(1, rt, 8, 0, 512)
            mm_piece(1, rt, 9, 0, 512)
        # slot 5: q1 distance-4 half
        norm_tp(5)
        for rt in range(8):
            mm_piece(1, rt, 10 if rt < 4 else 11, 0, 512)
        flush()
        emit_deferred_tr()

        _COLS["self"] = cols["self"]
        _COLS["off"] = cols["off"]

        nc.sync.dma_start(out=acc, in_=accw[:])


def _build():
    nc = bacc.Bacc(
        "TRN2",
        target_bir_lowering=False,
        debug=False,
        enable_asserts=False,
        num_devices=NCORES,
    )
    wind = nc.dram_tensor("wind", [NPAIR * WINDOW, DIM], F32, kind="ExternalInput").ap()
    gidx = nc.dram_tensor("gidx", [128, NPAIR * 128], I16, kind="ExternalInput").ap()
    acc = nc.dram_tensor("acc", [128, ACC_W], F32, kind="ExternalOutput").ap()
    with tile.TileContext(nc) as tc:
        _body(tc, wind, gidx, acc)
    nc.compile()
    return nc


_PROG = None


def _get_prog():
    global _PROG
    if _PROG is None:
        _PROG = _build()
    return _PROG


def _h(a):
    return 0 if a < 4 else 1


def _core_inputs(uid, pid, user_table, item_table, m):
    """per-core window tensor + int16 gather indices (sorted chunks)."""
    t = 0 if m < 4 else 1
    a1 = 2 * (m % 4)
    ids = [uid, pid][t]
    tab = [user_table, item_table][t]
    order = np.argsort(ids, kind="stable")
    svals = ids[order]

    slot_vals = []
    for i in range(NSLOT):
        c = (a1 + i) % 8
        v = svals[c * CHUNK : (c + 1) * CHUNK].copy()
        # distance-4 half-swap (complementary halves between core pairs)
        if i == 4 and _h(a1) == 1:
            v = np.concatenate([v[512:], v[:512]])
        if i == 5 and _h((a1 + 1) % 8) == 1:
            v = np.concatenate([v[512:], v[:512]])
        slot_vals.append(v)

    idx16 = np.zeros((128, NPAIR * 128), np.int16)
    bases = []
    for p in range(NPAIR):
        pairv = np.concatenate([slot_vals[2 * p], slot_vals[2 * p + 1]])
        base = min(int(pairv.min()), NROWS - WINDOW)
        assert int(pairv.max()) - base < WINDOW, (m, p)
        rel = (pairv - base).astype(np.int16)
        block = rel.reshape(128, 16).T  # idx i -> partition i%16, col i//16
        idx16[:, p * 128 : (p + 1) * 128] = np.tile(block, (8, 1))
        bases.append(base)

    wind = np.concatenate(
        [np.asarray(tab[b : b + WINDOW], dtype=np.float32) for b in bases], axis=0
    )
    return {"wind": np.ascontiguousarray(wind), "gidx": idx16}


def _make_in_maps(user_id, pos_id, user_table, item_table):
    uid = np.asarray(user_id).astype(np.int64)
    pid = np.asarray(pos_id).astype(np.int64)
    ut = np.asarray(user_table, dtype=np.float32)
    it = np.asarray(item_table, dtype=np.float32)
    return [_core_inputs(uid, pid, ut, it, m) for m in range(NCORES)]


def _host_align(user_id, pos_id, user_table, item_table):
    ue = np.asarray(user_table, dtype=np.float64)[np.asarray(user_id)]
    pe = np.asarray(item_table, dtype=np.float64)[np.asarray(pos_id)]
    un = ue / np.linalg.norm(ue, axis=1, keepdims=True)
    pn = pe / np.linalg.norm(pe, axis=1, keepdims=True)
    return 2.0 - (2.0 / B) * float(np.einsum("ij,ij->", un, pn))


def _finalize(accs, align):
    """accs: list of [128, ACC_W] per core -> scalar loss."""
    _get_prog()
    a = np.stack([np.asarray(x, dtype=np.float64) for x in accs])
    ns, no = _COLS["self"], _COLS["off"]
    s_self_u = a[0:4, :, SELF_COL0:ns].sum()
    s_off_u = a[0:4, :, OFF_COL0:no].sum()
    s_self_p = a[4:8, :, SELF_COL0:ns].sum()
    s_off_p = a[4:8, :, OFF_COL0:no].sum()
    npairs = B * (B - 1) // 2
    pair_u = s_off_u + (s_self_u - B) / 2.0
    pair_p = s_off_p + (s_self_p - B) / 2.0
    unif = 0.5 * (np.log(pair_u / npairs) + np.log(pair_p / npairs))
    return np.asarray(align + unif, dtype=np.float32)


def _run(in_maps, trace=False, **kw):
    nc = _get_prog()
    return bass_utils.run_bass_kernel_spmd(
        nc, in_maps, core_ids=list(range(NCORES)), trace=trace, **kw
    )


def kernel(user_id, pos_id, neg_id=None, user_table=None, item_table=None):
    in_maps = _make_in_maps(user_id, pos_id, user_table, item_table)
    align = _host_align(user_id, pos_id, user_table, item_table)
    res = _run(in_maps, trace=False)
    return _finalize([res.results[m]["acc"] for m in range(NCORES)], align)


def _install_profile_hook():
    """The image's antenv lacks axon_hooks; shim it so trace=True can reach
    the NTFF profiler in libaxon_pjrt.so (same mechanism trn_boot uses)."""
    import sys
    import types

    if "antenv.axon_hooks" in sys.modules:
        return
    import antenv
    from trn_agent_boot.trn_boot import _ntff_profile_via_ctypes

    mod = types.ModuleType("antenv.axon_hooks")
    holder = [None]
    mod.set_axon_ntff_profile_hook = lambda h: holder.__setitem__(0, h)
    mod.get_axon_ntff_profile_hook = lambda: holder[0]
    sys.modules["antenv.axon_hooks"] = mod
    antenv.axon_hooks = mod
    mod.set_axon_ntff_profile_hook(
        _ntff_profile_via_ctypes("/opt/axon/libaxon_pjrt.so")
    )
    bass_utils.upload_artifacts = lambda tmpdir: ""


def run_profiled(user_id, pos_id, neg_id=None, user_table=None, item_table=None, **kw):
    _install_profile_hook()
    in_maps = _make_in_maps(user_id, pos_id, user_table, item_table)
    align = _host_align(user_id, pos_id, user_table, item_table)
    res = _run(in_maps, trace=True, **kw)
    out = _finalize([res.results[m]["acc"] for m in range(NCORES)], align)
    return out, res


# revision 28
# speedup vs baseline: 1.0654x; 1.0654x over previous
"""DirectAU loss kernel for Trainium2, SPMD over 8 NeuronCores (v5).

Math (see reference):
  user_e = user_table[user_id]; pos_e = item_table[pos_id]   (B=8192, D=64)
  align  = mean_i ||un_i - pn_i||^2 = 2 - (2/B) sum_i <un_i, pn_i>
  unif(x)= log( (sum_{i<j} exp(-4 + 4 <xn_i, xn_j>)) / npairs )
  out    = align + 0.5*(unif(user_e) + unif(pos_e))

Strategy (v5, evolved from v4 at 130.7us):
  - Same chunk-pair coverage as v4: cores 0-3 own the user-table Gram,
    4-7 the pos one; each core owns 2 adjacent 1024-row chunks {a1,a1+1}
    (a1=2j) and multiplies them against chunks a1..a1+5, with the
    distance-4 blocks split in complementary halves between core pairs
    via the half-swapped gather order of slots 4/5.
  - Gathers use gpsimd.dma_gather (994ns + 0.34ns/row) instead of 48
    per-band indirect DMAs (994ns EACH): batch rows are SORTED by table
    id so each 1024-row chunk spans a ~12.5K id range that fits int16
    indices against a per-core 32K-row table window (the Gram is
    permutation invariant, so any chunking works). 4 issues (~5us on
    GpSimd) replace ~56 (~65us) - the v4 pipeline was gather-issue
    paced end to end.
  - The alignment term needs batch-paired rows of BOTH tables on one
    core, which sorting scrambles; it is 0.01% of the FLOPs and is
    folded into the host-side finalization (which already applies the
    closed-form log / diagonal corrections) in float64.
  - Normalization square/reduce/rsqrt run on the otherwise-idle Pool
    engine; DVE does only the fused multiply+fp8-cast and the
    PSUM->SBUF transpose copies (Pool has no PSUM port).
  - Exp drains are the wall (ACT: 0.833ns/col + ~0.5us/instr overhead
    = ~73us for all 66560 cols/core). A tunable fraction of the 2048-
    wide PSUM drain tiles is instead evaluated as a Schraudolph
    bitcast-exp on DVE (tensor_scalar i32 affine, calibrated to
    +2e-4 mean bias on the Gram distribution) with the f32-bitcast
    reduce on Pool, splitting the exp wall across three engines.
  - fp8-e4m3 DoubleRow Gram matmuls as v4 (PE-transposed [32,2,512]
    k-tile layout); matmuls emitted lhs-major in long uninterrupted
    streams so the PE p-state ramps instead of idling at 0.65GHz.
"""

import math

import numpy as np

import concourse.bacc as bacc
import concourse.bass as bass
import concourse.mybir as mybir
import concourse.tile as tile
from concourse import bass_utils
from concourse.masks import make_identity
from concourse.tile_rust import add_dep_helper

B = 8192
DIM = 64
NROWS = 100000
NCORES = 8
CHUNK = 1024
NSLOT = 6            # gathered main chunks per core (slots 0..5)
BANDS = NSLOT * 8    # 48 gather bands of 128 rows
NGRP = NSLOT * 2     # transpose groups of 4 bands (512 rows)
NPAIR = 3            # dma_gather windows (2 chunks each)
WINDOW = 32768       # rows per window (int16 index reach)
F32 = mybir.dt.float32
F8 = mybir.dt.float8e4
I16 = mybir.dt.int16
I32 = mybir.dt.int32

PSW = 2048           # PSUM work tile width (fp32)
ACC_W = 48
SELF_COL0 = 0        # self-tile accum columns (host removes diag double count)
OFF_COL0 = 4         # off-diagonal accum columns

# Schraudolph fast-exp for exp(4s-4): i32 = s*A + B_ , bitcast f32.
# B_ calibrated (C=-480000) for ~2e-4 mean bias over <xn_i,xn_j> ~ N(0,1/64).
A_SCH = float(np.float32(4.0 * (2.0 ** 23) / math.log(2.0)))
B_SCH = float(np.float32(127 * 2 ** 23 - 4.0 * (2.0 ** 23) / math.log(2.0) - 480000.0))

# off-drain k -> use DVE bitcast-exp instead of ACT exp (Pool cannot touch
# PSUM or run fp32 ALU ops, so the exp wall splits across ACT/DVE only)
def _dve_drain(k):
    return k >= 3 and (k % 4) == 1

_COLS = {"self": 0, "off": 0}  # filled at build; read by _finalize


def _emit_rsqrt(eng, pool, x_ap, out_ap, n, tag, order):
    """out = 1/sqrt(x) (bit-hack seed + 2 Newton steps) on engine `eng`."""
    MAGIC = 0x5F3759DF
    op = mybir.AluOpType
    ti = pool.tile([128, n], I32, tag=f"{tag}_ti", name=f"{tag}_ti")
    order(eng.tensor_scalar(
        out=ti[:], in0=x_ap.bitcast(I32), scalar1=1, scalar2=None,
        op0=op.logical_shift_right,
    ))
    yi = pool.tile([128, n], I32, tag=f"{tag}_yi", name=f"{tag}_yi")
    order(eng.tensor_scalar(
        out=yi[:], in0=ti[:], scalar1=-1, scalar2=None, op0=op.bitwise_xor
    ))
    order(eng.tensor_scalar(
        out=yi[:], in0=yi[:], scalar1=MAGIC + 1, scalar2=None, op0=op.add
    ))
    xh = pool.tile([128, n], F32, tag=f"{tag}_xh", name=f"{tag}_xh")
    order(eng.tensor_scalar(
        out=xh[:], in0=x_ap, scalar1=-0.5, scalar2=None, op0=op.mult
    ))
    cur = yi[:].bitcast(F32)
    for it in range(2):
        t2 = pool.tile([128, n], F32, tag=f"{tag}_t2", name=f"{tag}_t2")
        order(eng.tensor_tensor(out=t2[:], in0=cur, in1=cur, op=op.mult))
        order(eng.tensor_tensor(out=t2[:], in0=t2[:], in1=xh[:], op=op.mult))
        order(eng.tensor_scalar(
            out=t2[:], in0=t2[:], scalar1=1.5, scalar2=None, op0=op.add
        ))
        if it == 1:
            dst_ap = out_ap
        else:
            yt = pool.tile([128, n], F32, tag=f"{tag}_y", name=f"{tag}_y{it}")
            dst_ap = yt[:]
        order(eng.tensor_tensor(out=dst_ap, in0=cur, in1=t2[:], op=op.mult))
        cur = dst_ap


def _body(tc, wind, gidx, idn, acc):
    nc = tc.nc
    op = mybir.AluOpType
    DR = mybir.MatmulPerfMode.DoubleRow
    with (
        tc.tile_pool(name="persist", bufs=1) as P,
        tc.tile_pool(name="work", bufs=2) as W,
        tc.tile_pool(name="ps", bufs=2, space="PSUM") as PS,
    ):
        idx_sb = P.tile([128, NPAIR * 128], I16, tag="idx")

        accw = P.tile([128, ACC_W], F32, tag="accw")
        bias_o = P.tile([128, 1], F32, tag="bias_o")
        ident8 = P.tile([128, 128], F8, tag="ident8")

        # gathered rows, [128, band, DIM]: row i of slot c -> partition i%128,
        # band c*8 + i//128 (dma_gather's native layout)
        gath = P.tile([128, BANDS * DIM], F32, tag="gath")
        gath8 = P.tile([128, BANDS * DIM], F8, tag="gath8")
        # fp8 transposed layout: group g (4 bands = 512 rows) occupies cols
        # [g*1024, (g+1)*1024): [32 partitions, k-half h in {0,1}, 512 rows]
        xnT8 = P.tile([32, NGRP * 1024], F8, tag="xnT8")
        nsq = P.tile([128, BANDS], F32, tag="nsq")
        rinv = P.tile([128, BANDS], F32, tag="rinv")

        # queue-order pinning (the scheduler cost model mis-predicts gather
        # and PE readiness; pin each in-order engine to emission order)
        last = {"pe": None, "act": None, "dve": None, "pool": None}

        def _mk(key):
            def f(inst):
                if last[key] is not None:
                    add_dep_helper(inst.ins, last[key].ins, sync=False,
                                   reason=f"{key} order")
                last[key] = inst
                return inst
            return f

        pe_o, act_o, dve_o, pool_o = _mk("pe"), _mk("act"), _mk("dve"), _mk("pool")

        def gather(slot0, nslots):
            """one dma_gather for slots [slot0, slot0+nslots) out of window
            slot0//2 (idx data is window-relative)."""
            p = slot0 // 2
            n = nslots * CHUNK
            pool_o(nc.gpsimd.dma_gather(
                out_ap=gath[:, slot0 * 8 * DIM : (slot0 + nslots) * 8 * DIM]
                .rearrange("q (c d) -> q c d", d=DIM),
                in_ap=wind[p * WINDOW : (p + 1) * WINDOW, :],
                idxs_ap=idx_sb[:, slot0 * 64 : slot0 * 64 + n // 16],
                num_idxs=n,
                num_idxs_reg=n,
                elem_size=DIM,
            ))

        def setup_consts():
            warm = P.tile([128, 1], F32, tag="warm")
            act_o(nc.scalar.activation(
                out=warm[:], in_=bias_o[:],
                func=mybir.ActivationFunctionType.Exp,
            ))

        def norm_pool(c0, c1):
            """square + reduce + rsqrt on DVE for bands [c0, c1) (Pool's ALU
            is integer-only and has no PSUM/fp32 path)."""
            nb = c1 - c0
            g3 = gath[:, c0 * DIM : c1 * DIM].rearrange("p (c d) -> p c d", d=DIM)
            sq = W.tile([128, 16 * DIM], F32, tag="sq", name=f"sq{c0}")
            dve_o(nc.vector.tensor_tensor(
                out=sq[:, 0 : nb * DIM], in0=g3, in1=g3, op=op.mult))
            dve_o(nc.vector.tensor_reduce(
                out=nsq[:, c0:c1],
                in_=sq[:, 0 : nb * DIM].rearrange("p (c d) -> p c d", d=DIM),
                axis=mybir.AxisListType.X,
                op=op.add,
            ))
            _emit_rsqrt(nc.vector, W, nsq[:, c0:c1], rinv[:, c0:c1], nb,
                        f"rs{c0}", dve_o)

        def norm_mul(c0, c1):
            """fused normalize-multiply + fp8 cast on DVE."""
            nb = c1 - c0
            g3 = gath[:, c0 * DIM : c1 * DIM].rearrange("p (c d) -> p c d", d=DIM)
            r3 = (
                rinv[:, c0:c1]
                .rearrange("p (c o) -> p c o", o=1)
                .to_broadcast([128, nb, DIM])
            )
            g83 = gath8[:, c0 * DIM : c1 * DIM].rearrange("p (c d) -> p c d", d=DIM)
            dve_o(nc.vector.tensor_tensor(out=g83, in0=g3, in1=r3, op=op.mult))

        def transpose_group(g):
            """8 fp8 transposes (4 bands x 2 halves) -> [32,1024] PSUM, then
            DVE-copy the packed group into xnT8 (estep-2 PSUM constraint)."""
            pt8 = PS.tile([32, 2048], F8, tag="ps", name=f"tp{g}")
            for bi in range(4):
                c = g * 4 + bi
                for h in range(2):
                    s = 2 * (h * 512 + bi * 128)
                    pe_o(nc.tensor.transpose(
                        out=pt8[0:32, s : s + 256 : 2],
                        in_=gath8[:, c * DIM + h * 32 : c * DIM + (h + 1) * 32],
                        identity=ident8[:],
                    ))
            dve_o(nc.vector.tensor_copy(
                out=xnT8[:, g * 1024 : (g + 1) * 1024], in_=pt8[0:32, 0:2048:2]
            ))

        def rhs_ap(g, co, w):
            return xnT8[:, g * 1024 : (g + 1) * 1024].rearrange(
                "p (h c) -> p h c", h=2
            )[:, :, co : co + w]

        def lhs_ap(q, rt):
            return rhs_ap(q * 2 + rt // 4, (rt % 4) * 128, 128)

        # ---- rolling drain emitter: 2048-wide tiles, segments by kind ----
        st = {"tile": None, "fill": 0, "segs": [], "n": 0}
        cols = {"self": SELF_COL0, "off": OFF_COL0}
        offk = [0]
        dvek = [0]

        def mm_piece(q, rt, g, co, w, kind="off"):
            lhs = lhs_ap(q, rt)
            while w > 0:
                if st["tile"] is None:
                    st["tile"] = PS.tile(
                        [128, PSW], F32, tag="ps", name=f"mm{st['n']}"
                    )
                    st["n"] += 1
                take = min(w, PSW - st["fill"], 512 - st["fill"] % 512)
                pe_o(nc.tensor.matmul(
                    out=st["tile"][:, st["fill"] : st["fill"] + take],
                    lhsT=lhs,
                    rhs=rhs_ap(g, co, take),
                    start=True,
                    stop=True,
                    perf_mode=DR,
                ))
                if st["segs"] and st["segs"][-1][2] == kind \
                        and st["segs"][-1][1] == st["fill"]:
                    st["segs"][-1] = (st["segs"][-1][0], st["fill"] + take, kind)
                else:
                    st["segs"].append((st["fill"], st["fill"] + take, kind))
                st["fill"] += take
                co += take
                w -= take
                if st["fill"] == PSW:
                    flush()

        def drain_act(ap_in, col):
            act_o(nc.scalar.activation(
                out=ap_in,
                in_=ap_in,
                func=mybir.ActivationFunctionType.Exp,
                bias=bias_o[:],
                scale=4.0,
                accum_out=accw[:, col : col + 1],
            ))

        deferred_tr = []

        def drain_dve(ap_in, w, col):
            # convert PSUM->SBUF immediately (frees the PSUM slot for PE so
            # ACT never starves on the 2-slot rotation); the reduce of the
            # staged tile is deferred into DVE slack.
            k = dvek[0]
            dvek[0] += 1
            cv = P.tile([128, PSW], I32, tag=f"cv{k}", name=f"cv{k}")
            dve_o(nc.vector.tensor_scalar(
                out=cv[:, 0:w], in0=ap_in, scalar1=A_SCH, scalar2=B_SCH,
                op0=op.mult, op1=op.add,
            ))
            deferred_tr.append((cv, w, col))

        def emit_deferred_tr(n=None):
            todo = deferred_tr if n is None else deferred_tr[:n]
            for cv, w, col in todo:
                dve_o(nc.vector.tensor_reduce(
                    out=accw[:, col : col + 1],
                    in_=cv[:, 0:w].bitcast(F32),
                    axis=mybir.AxisListType.X,
                    op=op.add,
                ))
            del deferred_tr[: len(todo)]

        def flush():
            if st["fill"]:
                t = st["tile"]
                for (lo, hi, kind) in st["segs"]:
                    if kind == "self":
                        col = cols["self"]
                        cols["self"] += 1
                        drain_act(t[:, lo:hi], col)
                    else:
                        k = offk[0]
                        offk[0] += 1
                        col = cols["off"]
                        cols["off"] += 1
                        if _dve_drain(k):
                            drain_dve(t[:, lo:hi], hi - lo, col)
                        else:
                            drain_act(t[:, lo:hi], col)
            st["tile"] = None
            st["fill"] = 0
            st["segs"] = []
            # trickle one deferred reduce once it is a few tiles stale
            if len(deferred_tr) >= 3:
                emit_deferred_tr(1)

        def self_pass(q):
            for rt in range(8):
                mm_piece(q, rt, *(_lhs_loc(q, rt) + (128,)), kind="self")

        def _lhs_loc(q, rt):
            return (q * 2 + rt // 4, (rt % 4) * 128)

        def up_pass(q):
            # strict upper triangle of diag chunk q at 128-tile granularity
            for rt in range(8):
                s = (rt + 1) * 128
                for lo, hi in ((s, 512), (max(s, 512), 1024)):
                    if hi > lo:
                        mm_piece(q, rt, q * 2 + lo // 512, lo % 512, hi - lo)

        # ---- emission ----
        # NOTE: one dma_gather per 1024-row chunk. 2048-idx gathers emit 129
        # descriptors, one over the 128-deep SWDGE ring -> device lockup.
        # Gather descgen measures ~8.4ns/row of Pool-engine time (the cost
        # model's 0.34ns/desc is wrong for the gather ucode), so the ~52us
        # gather stream paces the kernel: the Pool queue carries NOTHING but
        # the six gathers (identity ships from host, memsets run on DVE) and
        # all downstream work is emitted in slot-availability order.
        nc.sync.dma_start(out=idx_sb[:], in_=gidx)
        nc.sync.dma_start(out=ident8[:], in_=idn)
        dve_o(nc.vector.memset(bias_o[:], -4.0))
        dve_o(nc.vector.memset(accw[:], 0.0))
        gather(0, 1)
        gather(1, 1)
        setup_consts()
        gather(2, 1)
        gather(3, 1)
        gather(4, 1)
        gather(5, 1)

        def norm_tp(slot):
            norm_pool(slot * 8, (slot + 1) * 8)
            norm_mul(slot * 8, (slot + 1) * 8)
            transpose_group(2 * slot)
            transpose_group(2 * slot + 1)

        # slot 0: q0 self + upper
        norm_tp(0)
        self_pass(0)
        up_pass(0)
        # slot 1: sibling block + q1 self + upper
        norm_tp(1)
        for rt in range(8):
            mm_piece(0, rt, 2, 0, 512)
            mm_piece(0, rt, 3, 0, 512)
        self_pass(1)
        up_pass(1)
        # slots 2,3: q0/q1 x slot
        for s in (2, 3):
            norm_tp(s)
            for q in (0, 1):
                for rt in range(8):
                    mm_piece(q, rt, 2 * s, 0, 512)
                    mm_piece(q, rt, 2 * s + 1, 0, 512)
        # slot 4: q0 distance-4 half (half-swapped) + q1 full
        norm_tp(4)
        for rt in range(8):
            mm_piece(0, rt, 8 if rt < 4 else 9, 0, 512)
        for rt in range(8):
            mm_piece(1, rt, 8, 0, 512)
            mm_piece(1, rt, 9, 0, 512)
        # slot 5: q1 distance-4 half
        norm_tp(5)
        for rt in range(8):
            mm_piece(1, rt, 10 if rt < 4 else 11, 0, 512)
        flush()
        emit_deferred_tr()

        _COLS["self"] = cols["self"]
        _COLS["off"] = cols["off"]

        nc.sync.dma_start(out=acc, in_=accw[:])


def _build():
    nc = bacc.Bacc(
        "TRN2",
        target_bir_lowering=False,
        debug=False,
        enable_asserts=False,
        num_devices=NCORES,
    )
    wind = nc.dram_tensor("wind", [NPAIR * WINDOW, DIM], F32, kind="ExternalInput").ap()
    gidx = nc.dram_tensor("gidx", [128, NPAIR * 128], I16, kind="ExternalInput").ap()
    idn = nc.dram_tensor("idn", [128, 128], F8, kind="ExternalInput").ap()
    acc = nc.dram_tensor("acc", [128, ACC_W], F32, kind="ExternalOutput").ap()
    with tile.TileContext(nc) as tc:
        _body(tc, wind, gidx, idn, acc)
    nc.compile()
    return nc


_PROG = None


def _get_prog():
    global _PROG
    if _PROG is None:
        _PROG = _build()
    return _PROG


def _h(a):
    return 0 if a < 4 else 1


def _core_inputs(uid, pid, user_table, item_table, m):
    """per-core window tensor + int16 gather indices (sorted chunks)."""
    t = 0 if m < 4 else 1
    a1 = 2 * (m % 4)
    ids = [uid, pid][t]
    tab = [user_table, item_table][t]
    order = np.argsort(ids, kind="stable")
    svals = ids[order]

    slot_vals = []
    for i in range(NSLOT):
        c = (a1 + i) % 8
        v = svals[c * CHUNK : (c + 1) * CHUNK].copy()
        # distance-4 half-swap (complementary halves between core pairs)
        if i == 4 and _h(a1) == 1:
            v = np.concatenate([v[512:], v[:512]])
        if i == 5 and _h((a1 + 1) % 8) == 1:
            v = np.concatenate([v[512:], v[:512]])
        slot_vals.append(v)

    idx16 = np.zeros((128, NPAIR * 128), np.int16)
    bases = []
    for p in range(NPAIR):
        pairv = np.concatenate([slot_vals[2 * p], slot_vals[2 * p + 1]])
        base = min(int(pairv.min()), NROWS - WINDOW)
        assert int(pairv.max()) - base < WINDOW, (m, p)
        rel = (pairv - base).astype(np.int16)
        block = rel.reshape(128, 16).T  # idx i -> partition i%16, col i//16
        idx16[:, p * 128 : (p + 1) * 128] = np.tile(block, (8, 1))
        bases.append(base)

    wind = np.concatenate(
        [np.asarray(tab[b : b + WINDOW], dtype=np.float32) for b in bases], axis=0
    )
    import ml_dtypes
    idn = np.eye(128, dtype=np.float32).astype(ml_dtypes.float8_e4m3)
    return {"wind": np.ascontiguousarray(wind), "gidx": idx16, "idn": idn}


def _make_in_maps(user_id, pos_id, user_table, item_table):
    uid = np.asarray(user_id).astype(np.int64)
    pid = np.asarray(pos_id).astype(np.int64)
    ut = np.asarray(user_table, dtype=np.float32)
    it = np.asarray(item_table, dtype=np.float32)
    return [_core_inputs(uid, pid, ut, it, m) for m in range(NCORES)]


def _host_align(user_id, pos_id, user_table, item_table):
    ue = np.asarray(user_table, dtype=np.float64)[np.asarray(user_id)]
    pe = np.asarray(item_table, dtype=np.float64)[np.asarray(pos_id)]
    un = ue / np.linalg.norm(ue, axis=1, keepdims=True)
    pn = pe / np.linalg.norm(pe, axis=1, keepdims=True)
    return 2.0 - (2.0 / B) * float(np.einsum("ij,ij->", un, pn))


def _finalize(accs, align):
    """accs: list of [128, ACC_W] per core -> scalar loss."""
    _get_prog()
    a = np.stack([np.asarray(x, dtype=np.float64) for x in accs])
    ns, no = _COLS["self"], _COLS["off"]
    s_self_u = a[0:4, :, SELF_COL0:ns].sum()
    s_off_u = a[0:4, :, OFF_COL0:no].sum()
    s_self_p = a[4:8, :, SELF_COL0:ns].sum()
    s_off_p = a[4:8, :, OFF_COL0:no].sum()
    npairs = B * (B - 1) // 2
    pair_u = s_off_u + (s_self_u - B) / 2.0
    pair_p = s_off_p + (s_self_p - B) / 2.0
    unif = 0.5 * (np.log(pair_u / npairs) + np.log(pair_p / npairs))
    return np.asarray(align + unif, dtype=np.float32)


def _run(in_maps, trace=False, **kw):
    nc = _get_prog()
    return bass_utils.run_bass_kernel_spmd(
        nc, in_maps, core_ids=list(range(NCORES)), trace=trace, **kw
    )


def kernel(user_id, pos_id, neg_id=None, user_table=None, item_table=None):
    in_maps = _make_in_maps(user_id, pos_id, user_table, item_table)
    align = _host_align(user_id, pos_id, user_table, item_table)
    res = _run(in_maps, trace=False)
    return _finalize([res.results[m]["acc"] for m in range(NCORES)], align)


def _install_profile_hook():
    """The image's antenv lacks axon_hooks; shim it so trace=True can reach
    the NTFF profiler in libaxon_pjrt.so (same mechanism trn_boot uses)."""
    import sys
    import types

    if "antenv.axon_hooks" in sys.modules:
        return
    import antenv
    from trn_agent_boot.trn_boot import _ntff_profile_via_ctypes

    mod = types.ModuleType("antenv.axon_hooks")
    holder = [None]
    mod.set_axon_ntff_profile_hook = lambda h: holder.__setitem__(0, h)
    mod.get_axon_ntff_profile_hook = lambda: holder[0]
    sys.modules["antenv.axon_hooks"] = mod
    antenv.axon_hooks = mod
    mod.set_axon_ntff_profile_hook(
        _ntff_profile_via_ctypes("/opt/axon/libaxon_pjrt.so")
    )
    bass_utils.upload_artifacts = lambda tmpdir: ""


def run_profiled(user_id, pos_id, neg_id=None, user_table=None, item_table=None, **kw):
    _install_profile_hook()
    in_maps = _make_in_maps(user_id, pos_id, user_table, item_table)
    align = _host_align(user_id, pos_id, user_table, item_table)
    res = _run(in_maps, trace=True, **kw)
    out = _finalize([res.results[m]["acc"] for m in range(NCORES)], align)
    return out, res


# revision 29
# speedup vs baseline: 1.1226x; 1.0537x over previous
"""DirectAU loss kernel for Trainium2, SPMD over 8 NeuronCores (v5).

Math (see reference):
  user_e = user_table[user_id]; pos_e = item_table[pos_id]   (B=8192, D=64)
  align  = mean_i ||un_i - pn_i||^2 = 2 - (2/B) sum_i <un_i, pn_i>
  unif(x)= log( (sum_{i<j} exp(-4 + 4 <xn_i, xn_j>)) / npairs )
  out    = align + 0.5*(unif(user_e) + unif(pos_e))

Strategy (v5, evolved from v4 at 130.7us):
  - Same chunk-pair coverage as v4: cores 0-3 own the user-table Gram,
    4-7 the pos one; each core owns 2 adjacent 1024-row chunks {a1,a1+1}
    (a1=2j) and multiplies them against chunks a1..a1+5, with the
    distance-4 blocks split in complementary halves between core pairs
    via the half-swapped gather order of slots 4/5.
  - Gathers use gpsimd.dma_gather (994ns + 0.34ns/row) instead of 48
    per-band indirect DMAs (994ns EACH): batch rows are SORTED by table
    id so each 1024-row chunk spans a ~12.5K id range that fits int16
    indices against a per-core 32K-row table window (the Gram is
    permutation invariant, so any chunking works). 4 issues (~5us on
    GpSimd) replace ~56 (~65us) - the v4 pipeline was gather-issue
    paced end to end.
  - The alignment term needs batch-paired rows of BOTH tables on one
    core, which sorting scrambles; it is 0.01% of the FLOPs and is
    folded into the host-side finalization (which already applies the
    closed-form log / diagonal corrections) in float64.
  - Normalization square/reduce/rsqrt run on the otherwise-idle Pool
    engine; DVE does only the fused multiply+fp8-cast and the
    PSUM->SBUF transpose copies (Pool has no PSUM port).
  - Exp drains are the wall (ACT: 0.833ns/col + ~0.5us/instr overhead
    = ~73us for all 66560 cols/core). A tunable fraction of the 2048-
    wide PSUM drain tiles is instead evaluated as a Schraudolph
    bitcast-exp on DVE (tensor_scalar i32 affine, calibrated to
    +2e-4 mean bias on the Gram distribution) with the f32-bitcast
    reduce on Pool, splitting the exp wall across three engines.
  - fp8-e4m3 DoubleRow Gram matmuls as v4 (PE-transposed [32,2,512]
    k-tile layout); matmuls emitted lhs-major in long uninterrupted
    streams so the PE p-state ramps instead of idling at 0.65GHz.
"""

import math

import numpy as np

import concourse.bacc as bacc
import concourse.bass as bass
import concourse.mybir as mybir
import concourse.tile as tile
from concourse import bass_utils
from concourse.masks import make_identity
from concourse.tile_rust import add_dep_helper

B = 8192
DIM = 64
NROWS = 100000
NCORES = 8
CHUNK = 1024
NSLOT = 6            # gathered main chunks per core (slots 0..5)
BANDS = NSLOT * 8    # 48 gather bands of 128 rows
NGRP = NSLOT * 2     # transpose groups of 4 bands (512 rows)
NPAIR = 3            # dma_gather windows (2 chunks each)
WINDOW = 32768       # rows per window (int16 index reach)
F32 = mybir.dt.float32
F8 = mybir.dt.float8e4
I16 = mybir.dt.int16
I32 = mybir.dt.int32

PSW = 2048           # PSUM work tile width (fp32)
ACC_W = 48
SELF_COL0 = 0        # self-tile accum columns (host removes diag double count)
OFF_COL0 = 4         # off-diagonal accum columns

# Schraudolph fast-exp for exp(4s-4): i32 = s*A + B_ , bitcast f32.
# B_ calibrated (C=-480000) for ~2e-4 mean bias over <xn_i,xn_j> ~ N(0,1/64).
A_SCH = float(np.float32(4.0 * (2.0 ** 23) / math.log(2.0)))
B_SCH = float(np.float32(127 * 2 ** 23 - 4.0 * (2.0 ** 23) / math.log(2.0) - 480000.0))

# off-drain k -> use DVE bitcast-exp instead of ACT exp (Pool cannot touch
# PSUM or run fp32 ALU ops, so the exp wall splits across ACT/DVE only).
# With 2 PSUM work slots a third concurrent consumer stalls the rotation,
# so DVE only co-drains the post-gather tail.
def _dve_drain(k):
    return k >= 28 and (k % 2) == 1

_COLS = {"self": 0, "off": 0}  # filled at build; read by _finalize


def _emit_rsqrt(eng, pool, x_ap, out_ap, n, tag, order):
    """out = 1/sqrt(x) (bit-hack seed + 2 Newton steps) on engine `eng`."""
    MAGIC = 0x5F3759DF
    op = mybir.AluOpType
    ti = pool.tile([128, n], I32, tag=f"{tag}_ti", name=f"{tag}_ti")
    order(eng.tensor_scalar(
        out=ti[:], in0=x_ap.bitcast(I32), scalar1=1, scalar2=None,
        op0=op.logical_shift_right,
    ))
    yi = pool.tile([128, n], I32, tag=f"{tag}_yi", name=f"{tag}_yi")
    order(eng.tensor_scalar(
        out=yi[:], in0=ti[:], scalar1=-1, scalar2=None, op0=op.bitwise_xor
    ))
    order(eng.tensor_scalar(
        out=yi[:], in0=yi[:], scalar1=MAGIC + 1, scalar2=None, op0=op.add
    ))
    xh = pool.tile([128, n], F32, tag=f"{tag}_xh", name=f"{tag}_xh")
    order(eng.tensor_scalar(
        out=xh[:], in0=x_ap, scalar1=-0.5, scalar2=None, op0=op.mult
    ))
    cur = yi[:].bitcast(F32)
    for it in range(2):
        t2 = pool.tile([128, n], F32, tag=f"{tag}_t2", name=f"{tag}_t2")
        order(eng.tensor_tensor(out=t2[:], in0=cur, in1=cur, op=op.mult))
        order(eng.tensor_tensor(out=t2[:], in0=t2[:], in1=xh[:], op=op.mult))
        order(eng.tensor_scalar(
            out=t2[:], in0=t2[:], scalar1=1.5, scalar2=None, op0=op.add
        ))
        if it == 1:
            dst_ap = out_ap
        else:
            yt = pool.tile([128, n], F32, tag=f"{tag}_y", name=f"{tag}_y{it}")
            dst_ap = yt[:]
        order(eng.tensor_tensor(out=dst_ap, in0=cur, in1=t2[:], op=op.mult))
        cur = dst_ap


def _body(tc, wind, gidx, idn, acc):
    nc = tc.nc
    op = mybir.AluOpType
    DR = mybir.MatmulPerfMode.DoubleRow
    with (
        tc.tile_pool(name="persist", bufs=1) as P,
        tc.tile_pool(name="work", bufs=2) as W,
        tc.tile_pool(name="ps", bufs=2, space="PSUM") as PS,
    ):
        idx_sb = P.tile([128, NPAIR * 128], I16, tag="idx")

        accw = P.tile([128, ACC_W], F32, tag="accw")
        bias_o = P.tile([128, 1], F32, tag="bias_o")
        ident8 = P.tile([128, 128], F8, tag="ident8")

        # gathered rows, [128, band, DIM]: row i of slot c -> partition i%128,
        # band c*8 + i//128 (dma_gather's native layout)
        gath = P.tile([128, BANDS * DIM], F32, tag="gath")
        gath8 = P.tile([128, BANDS * DIM], F8, tag="gath8")
        # fp8 transposed layout: group g (4 bands = 512 rows) occupies cols
        # [g*1024, (g+1)*1024): [32 partitions, k-half h in {0,1}, 512 rows]
        xnT8 = P.tile([32, NGRP * 1024], F8, tag="xnT8")
        nsq = P.tile([128, BANDS], F32, tag="nsq")
        rinv = P.tile([128, BANDS], F32, tag="rinv")

        # queue-order pinning (the scheduler cost model mis-predicts gather
        # and PE readiness; pin each in-order engine to emission order)
        last = {"pe": None, "act": None, "dve": None, "pool": None}

        def _mk(key):
            def f(inst):
                if last[key] is not None:
                    add_dep_helper(inst.ins, last[key].ins, sync=False,
                                   reason=f"{key} order")
                last[key] = inst
                return inst
            return f

        pe_o, act_o, dve_o, pool_o = _mk("pe"), _mk("act"), _mk("dve"), _mk("pool")

        def gather(slot0, nslots):
            """one dma_gather for slots [slot0, slot0+nslots) out of window
            slot0//2 (idx data is window-relative)."""
            p = slot0 // 2
            n = nslots * CHUNK
            pool_o(nc.gpsimd.dma_gather(
                out_ap=gath[:, slot0 * 8 * DIM : (slot0 + nslots) * 8 * DIM]
                .rearrange("q (c d) -> q c d", d=DIM),
                in_ap=wind[p * WINDOW : (p + 1) * WINDOW, :],
                idxs_ap=idx_sb[:, slot0 * 64 : slot0 * 64 + n // 16],
                num_idxs=n,
                num_idxs_reg=n,
                elem_size=DIM,
            ))

        def setup_consts():
            warm = P.tile([128, 1], F32, tag="warm")
            act_o(nc.scalar.activation(
                out=warm[:], in_=bias_o[:],
                func=mybir.ActivationFunctionType.Exp,
            ))

        def norm_pool(c0, c1):
            """square + reduce + rsqrt on DVE for bands [c0, c1) (Pool's ALU
            is integer-only and has no PSUM/fp32 path)."""
            nb = c1 - c0
            g3 = gath[:, c0 * DIM : c1 * DIM].rearrange("p (c d) -> p c d", d=DIM)
            sq = W.tile([128, 16 * DIM], F32, tag="sq", name=f"sq{c0}")
            dve_o(nc.vector.tensor_tensor(
                out=sq[:, 0 : nb * DIM], in0=g3, in1=g3, op=op.mult))
            dve_o(nc.vector.tensor_reduce(
                out=nsq[:, c0:c1],
                in_=sq[:, 0 : nb * DIM].rearrange("p (c d) -> p c d", d=DIM),
                axis=mybir.AxisListType.X,
                op=op.add,
            ))
            _emit_rsqrt(nc.vector, W, nsq[:, c0:c1], rinv[:, c0:c1], nb,
                        f"rs{c0}", dve_o)

        def norm_mul(c0, c1):
            """fused normalize-multiply + fp8 cast on DVE."""
            nb = c1 - c0
            g3 = gath[:, c0 * DIM : c1 * DIM].rearrange("p (c d) -> p c d", d=DIM)
            r3 = (
                rinv[:, c0:c1]
                .rearrange("p (c o) -> p c o", o=1)
                .to_broadcast([128, nb, DIM])
            )
            g83 = gath8[:, c0 * DIM : c1 * DIM].rearrange("p (c d) -> p c d", d=DIM)
            dve_o(nc.vector.tensor_tensor(out=g83, in0=g3, in1=r3, op=op.mult))

        def transpose_group(g):
            """8 fp8 transposes (4 bands x 2 halves) -> [32,1024] PSUM, then
            DVE-copy the packed group into xnT8 (estep-2 PSUM constraint)."""
            pt8 = PS.tile([32, 2048], F8, tag="ps", name=f"tp{g}")
            for bi in range(4):
                c = g * 4 + bi
                for h in range(2):
                    s = 2 * (h * 512 + bi * 128)
                    pe_o(nc.tensor.transpose(
                        out=pt8[0:32, s : s + 256 : 2],
                        in_=gath8[:, c * DIM + h * 32 : c * DIM + (h + 1) * 32],
                        identity=ident8[:],
                    ))
            dve_o(nc.vector.tensor_copy(
                out=xnT8[:, g * 1024 : (g + 1) * 1024], in_=pt8[0:32, 0:2048:2]
            ))

        def rhs_ap(g, co, w):
            return xnT8[:, g * 1024 : (g + 1) * 1024].rearrange(
                "p (h c) -> p h c", h=2
            )[:, :, co : co + w]

        def lhs_ap(q, rt):
            return rhs_ap(q * 2 + rt // 4, (rt % 4) * 128, 128)

        # ---- rolling drain emitter: 2048-wide tiles, segments by kind ----
        st = {"tile": None, "fill": 0, "segs": [], "n": 0}
        cols = {"self": SELF_COL0, "off": OFF_COL0}
        offk = [0]
        dvek = [0]

        def mm_piece(q, rt, g, co, w, kind="off"):
            lhs = lhs_ap(q, rt)
            while w > 0:
                if st["tile"] is None:
                    st["tile"] = PS.tile(
                        [128, PSW], F32, tag="ps", name=f"mm{st['n']}"
                    )
                    st["n"] += 1
                take = min(w, PSW - st["fill"], 512 - st["fill"] % 512)
                pe_o(nc.tensor.matmul(
                    out=st["tile"][:, st["fill"] : st["fill"] + take],
                    lhsT=lhs,
                    rhs=rhs_ap(g, co, take),
                    start=True,
                    stop=True,
                    perf_mode=DR,
                ))
                if st["segs"] and st["segs"][-1][2] == kind \
                        and st["segs"][-1][1] == st["fill"]:
                    st["segs"][-1] = (st["segs"][-1][0], st["fill"] + take, kind)
                else:
                    st["segs"].append((st["fill"], st["fill"] + take, kind))
                st["fill"] += take
                co += take
                w -= take
                if st["fill"] == PSW:
                    flush()

        def drain_act(ap_in, col):
            act_o(nc.scalar.activation(
                out=ap_in,
                in_=ap_in,
                func=mybir.ActivationFunctionType.Exp,
                bias=bias_o[:],
                scale=4.0,
                accum_out=accw[:, col : col + 1],
            ))

        deferred_tr = []

        def drain_dve(ap_in, w, col):
            # convert PSUM->SBUF immediately (frees the PSUM slot for PE so
            # ACT never starves on the 2-slot rotation); the reduce of the
            # staged tile is deferred into DVE slack.
            k = dvek[0]
            dvek[0] += 1
            cv = P.tile([128, PSW], I32, tag=f"cv{k}", name=f"cv{k}")
            dve_o(nc.vector.tensor_scalar(
                out=cv[:, 0:w], in0=ap_in, scalar1=A_SCH, scalar2=B_SCH,
                op0=op.mult, op1=op.add,
            ))
            deferred_tr.append((cv, w, col))

        def emit_deferred_tr(n=None):
            todo = deferred_tr if n is None else deferred_tr[:n]
            for cv, w, col in todo:
                dve_o(nc.vector.tensor_reduce(
                    out=accw[:, col : col + 1],
                    in_=cv[:, 0:w].bitcast(F32),
                    axis=mybir.AxisListType.X,
                    op=op.add,
                ))
            del deferred_tr[: len(todo)]

        def flush():
            if st["fill"]:
                t = st["tile"]
                for (lo, hi, kind) in st["segs"]:
                    if kind == "self":
                        col = cols["self"]
                        cols["self"] += 1
                        drain_act(t[:, lo:hi], col)
                    else:
                        k = offk[0]
                        offk[0] += 1
                        col = cols["off"]
                        cols["off"] += 1
                        if _dve_drain(k):
                            drain_dve(t[:, lo:hi], hi - lo, col)
                        else:
                            drain_act(t[:, lo:hi], col)
            st["tile"] = None
            st["fill"] = 0
            st["segs"] = []
            # trickle one deferred reduce once it is a few tiles stale
            if len(deferred_tr) >= 3:
                emit_deferred_tr(1)

        def self_pass(q):
            for rt in range(8):
                mm_piece(q, rt, *(_lhs_loc(q, rt) + (128,)), kind="self")

        def _lhs_loc(q, rt):
            return (q * 2 + rt // 4, (rt % 4) * 128)

        def up_pass(q):
            # strict upper triangle of diag chunk q at 128-tile granularity
            for rt in range(8):
                s = (rt + 1) * 128
                for lo, hi in ((s, 512), (max(s, 512), 1024)):
                    if hi > lo:
                        mm_piece(q, rt, q * 2 + lo // 512, lo % 512, hi - lo)

        # ---- emission ----
        # NOTE: one dma_gather per 1024-row chunk. 2048-idx gathers emit 129
        # descriptors, one over the 128-deep SWDGE ring -> device lockup.
        # Gather descgen measures ~8.4ns/row of Pool-engine time (the cost
        # model's 0.34ns/desc is wrong for the gather ucode), so the ~52us
        # gather stream paces the kernel: the Pool queue carries NOTHING but
        # the six gathers (identity ships from host, memsets run on DVE) and
        # all downstream work is emitted in slot-availability order.
        nc.sync.dma_start(out=idx_sb[:], in_=gidx)
        nc.sync.dma_start(out=ident8[:], in_=idn)
        dve_o(nc.vector.memset(bias_o[:], -4.0))
        dve_o(nc.vector.memset(accw[:], 0.0))
        gather(0, 1)
        gather(1, 1)
        setup_consts()
        gather(2, 1)
        gather(3, 1)
        gather(4, 1)
        gather(5, 1)

        def norm_tp(slot):
            norm_pool(slot * 8, (slot + 1) * 8)
            norm_mul(slot * 8, (slot + 1) * 8)
            transpose_group(2 * slot)
            transpose_group(2 * slot + 1)

        # slot 0: q0 self + upper
        norm_tp(0)
        self_pass(0)
        up_pass(0)
        # slot 1: sibling block + q1 self + upper
        norm_tp(1)
        for rt in range(8):
            mm_piece(0, rt, 2, 0, 512)
            mm_piece(0, rt, 3, 0, 512)
        self_pass(1)
        up_pass(1)
        # slots 2,3: q0/q1 x slot
        for s in (2, 3):
            norm_tp(s)
            for q in (0, 1):
                for rt in range(8):
                    mm_piece(q, rt, 2 * s, 0, 512)
                    mm_piece(q, rt, 2 * s + 1, 0, 512)
        # slot 4: q0 distance-4 half (half-swapped) + q1 full
        norm_tp(4)
        for rt in range(8):
            mm_piece(0, rt, 8 if rt < 4 else 9, 0, 512)
        for rt in range(8):
            mm_piece(1, rt, 8, 0, 512)
            mm_piece(1, rt, 9, 0, 512)
        # slot 5: q1 distance-4 half
        norm_tp(5)
        for rt in range(8):
            mm_piece(1, rt, 10 if rt < 4 else 11, 0, 512)
        flush()
        emit_deferred_tr()

        _COLS["self"] = cols["self"]
        _COLS["off"] = cols["off"]

        nc.sync.dma_start(out=acc, in_=accw[:])


def _build():
    nc = bacc.Bacc(
        "TRN2",
        target_bir_lowering=False,
        debug=False,
        enable_asserts=False,
        num_devices=NCORES,
    )
    wind = nc.dram_tensor("wind", [NPAIR * WINDOW, DIM], F32, kind="ExternalInput").ap()
    gidx = nc.dram_tensor("gidx", [128, NPAIR * 128], I16, kind="ExternalInput").ap()
    idn = nc.dram_tensor("idn", [128, 128], F8, kind="ExternalInput").ap()
    acc = nc.dram_tensor("acc", [128, ACC_W], F32, kind="ExternalOutput").ap()
    with tile.TileContext(nc) as tc:
        _body(tc, wind, gidx, idn, acc)
    nc.compile()
    return nc


_PROG = None


def _get_prog():
    global _PROG
    if _PROG is None:
        _PROG = _build()
    return _PROG


def _h(a):
    return 0 if a < 4 else 1


def _core_inputs(uid, pid, user_table, item_table, m):
    """per-core window tensor + int16 gather indices (sorted chunks)."""
    t = 0 if m < 4 else 1
    a1 = 2 * (m % 4)
    ids = [uid, pid][t]
    tab = [user_table, item_table][t]
    order = np.argsort(ids, kind="stable")
    svals = ids[order]

    slot_vals = []
    for i in range(NSLOT):
        c = (a1 + i) % 8
        v = svals[c * CHUNK : (c + 1) * CHUNK].copy()
        # distance-4 half-swap (complementary halves between core pairs)
        if i == 4 and _h(a1) == 1:
            v = np.concatenate([v[512:], v[:512]])
        if i == 5 and _h((a1 + 1) % 8) == 1:
            v = np.concatenate([v[512:], v[:512]])
        slot_vals.append(v)

    idx16 = np.zeros((128, NPAIR * 128), np.int16)
    bases = []
    for p in range(NPAIR):
        pairv = np.concatenate([slot_vals[2 * p], slot_vals[2 * p + 1]])
        base = min(int(pairv.min()), NROWS - WINDOW)
        assert int(pairv.max()) - base < WINDOW, (m, p)
        rel = (pairv - base).astype(np.int16)
        block = rel.reshape(128, 16).T  # idx i -> partition i%16, col i//16
        idx16[:, p * 128 : (p + 1) * 128] = np.tile(block, (8, 1))
        bases.append(base)

    wind = np.concatenate(
        [np.asarray(tab[b : b + WINDOW], dtype=np.float32) for b in bases], axis=0
    )
    import ml_dtypes
    idn = np.eye(128, dtype=np.float32).astype(ml_dtypes.float8_e4m3)
    return {"wind": np.ascontiguousarray(wind), "gidx": idx16, "idn": idn}


def _make_in_maps(user_id, pos_id, user_table, item_table):
    uid = np.asarray(user_id).astype(np.int64)
    pid = np.asarray(pos_id).astype(np.int64)
    ut = np.asarray(user_table, dtype=np.float32)
    it = np.asarray(item_table, dtype=np.float32)
    return [_core_inputs(uid, pid, ut, it, m) for m in range(NCORES)]


def _host_align(user_id, pos_id, user_table, item_table):
    ue = np.asarray(user_table, dtype=np.float64)[np.asarray(user_id)]
    pe = np.asarray(item_table, dtype=np.float64)[np.asarray(pos_id)]
    un = ue / np.linalg.norm(ue, axis=1, keepdims=True)
    pn = pe / np.linalg.norm(pe, axis=1, keepdims=True)
    return 2.0 - (2.0 / B) * float(np.einsum("ij,ij->", un, pn))


def _finalize(accs, align):
    """accs: list of [128, ACC_W] per core -> scalar loss."""
    _get_prog()
    a = np.stack([np.asarray(x, dtype=np.float64) for x in accs])
    ns, no = _COLS["self"], _COLS["off"]
    s_self_u = a[0:4, :, SELF_COL0:ns].sum()
    s_off_u = a[0:4, :, OFF_COL0:no].sum()
    s_self_p = a[4:8, :, SELF_COL0:ns].sum()
    s_off_p = a[4:8, :, OFF_COL0:no].sum()
    npairs = B * (B - 1) // 2
    pair_u = s_off_u + (s_self_u - B) / 2.0
    pair_p = s_off_p + (s_self_p - B) / 2.0
    unif = 0.5 * (np.log(pair_u / npairs) + np.log(pair_p / npairs))
    return np.asarray(align + unif, dtype=np.float32)


def _run(in_maps, trace=False, **kw):
    nc = _get_prog()
    return bass_utils.run_bass_kernel_spmd(
        nc, in_maps, core_ids=list(range(NCORES)), trace=trace, **kw
    )


def kernel(user_id, pos_id, neg_id=None, user_table=None, item_table=None):
    in_maps = _make_in_maps(user_id, pos_id, user_table, item_table)
    align = _host_align(user_id, pos_id, user_table, item_table)
    res = _run(in_maps, trace=False)
    return _finalize([res.results[m]["acc"] for m in range(NCORES)], align)


def _install_profile_hook():
    """The image's antenv lacks axon_hooks; shim it so trace=True can reach
    the NTFF profiler in libaxon_pjrt.so (same mechanism trn_boot uses)."""
    import sys
    import types

    if "antenv.axon_hooks" in sys.modules:
        return
    import antenv
    from trn_agent_boot.trn_boot import _ntff_profile_via_ctypes

    mod = types.ModuleType("antenv.axon_hooks")
    holder = [None]
    mod.set_axon_ntff_profile_hook = lambda h: holder.__setitem__(0, h)
    mod.get_axon_ntff_profile_hook = lambda: holder[0]
    sys.modules["antenv.axon_hooks"] = mod
    antenv.axon_hooks = mod
    mod.set_axon_ntff_profile_hook(
        _ntff_profile_via_ctypes("/opt/axon/libaxon_pjrt.so")
    )
    bass_utils.upload_artifacts = lambda tmpdir: ""


def run_profiled(user_id, pos_id, neg_id=None, user_table=None, item_table=None, **kw):
    _install_profile_hook()
    in_maps = _make_in_maps(user_id, pos_id, user_table, item_table)
    align = _host_align(user_id, pos_id, user_table, item_table)
    res = _run(in_maps, trace=True, **kw)
    out = _finalize([res.results[m]["acc"] for m in range(NCORES)], align)
    return out, res


# revision 31
# speedup vs baseline: 1.1579x; 1.0315x over previous
"""DirectAU loss kernel for Trainium2, SPMD over 8 NeuronCores (v5).

Math (see reference):
  user_e = user_table[user_id]; pos_e = item_table[pos_id]   (B=8192, D=64)
  align  = mean_i ||un_i - pn_i||^2 = 2 - (2/B) sum_i <un_i, pn_i>
  unif(x)= log( (sum_{i<j} exp(-4 + 4 <xn_i, xn_j>)) / npairs )
  out    = align + 0.5*(unif(user_e) + unif(pos_e))

Strategy (v5, evolved from v4 at 130.7us):
  - Same chunk-pair coverage as v4: cores 0-3 own the user-table Gram,
    4-7 the pos one; each core owns 2 adjacent 1024-row chunks {a1,a1+1}
    (a1=2j) and multiplies them against chunks a1..a1+5, with the
    distance-4 blocks split in complementary halves between core pairs
    via the half-swapped gather order of slots 4/5.
  - Gathers use gpsimd.dma_gather (994ns + 0.34ns/row) instead of 48
    per-band indirect DMAs (994ns EACH): batch rows are SORTED by table
    id so each 1024-row chunk spans a ~12.5K id range that fits int16
    indices against a per-core 32K-row table window (the Gram is
    permutation invariant, so any chunking works). 4 issues (~5us on
    GpSimd) replace ~56 (~65us) - the v4 pipeline was gather-issue
    paced end to end.
  - The alignment term needs batch-paired rows of BOTH tables on one
    core, which sorting scrambles; it is 0.01% of the FLOPs and is
    folded into the host-side finalization (which already applies the
    closed-form log / diagonal corrections) in float64.
  - Normalization square/reduce/rsqrt run on the otherwise-idle Pool
    engine; DVE does only the fused multiply+fp8-cast and the
    PSUM->SBUF transpose copies (Pool has no PSUM port).
  - Exp drains are the wall (ACT: 0.833ns/col + ~0.5us/instr overhead
    = ~73us for all 66560 cols/core). A tunable fraction of the 2048-
    wide PSUM drain tiles is instead evaluated as a Schraudolph
    bitcast-exp on DVE (tensor_scalar i32 affine, calibrated to
    +2e-4 mean bias on the Gram distribution) with the f32-bitcast
    reduce on Pool, splitting the exp wall across three engines.
  - fp8-e4m3 DoubleRow Gram matmuls as v4 (PE-transposed [32,2,512]
    k-tile layout); matmuls emitted lhs-major in long uninterrupted
    streams so the PE p-state ramps instead of idling at 0.65GHz.
"""

import math

import numpy as np

import concourse.bacc as bacc
import concourse.bass as bass
import concourse.mybir as mybir
import concourse.tile as tile
from concourse import bass_utils
from concourse.masks import make_identity
from concourse.tile_rust import add_dep_helper

B = 8192
DIM = 64
NROWS = 100000
NCORES = 8
CHUNK = 1024
NSLOT = 6            # gathered main chunks per core (slots 0..5)
BANDS = NSLOT * 8    # 48 gather bands of 128 rows
NGRP = NSLOT * 2     # transpose groups of 4 bands (512 rows)
NPAIR = 3            # dma_gather windows (2 chunks each)
WINDOW = 32768       # rows per window (int16 index reach)
F32 = mybir.dt.float32
F8 = mybir.dt.float8e4
I16 = mybir.dt.int16
I32 = mybir.dt.int32

PSW = 2048           # PSUM work tile width (fp32)
ACC_W = 48
SELF_COL0 = 0        # self-tile accum columns (host removes diag double count)
OFF_COL0 = 4         # off-diagonal accum columns

# Schraudolph fast-exp for exp(4s-4): i32 = s*A + B_ , bitcast f32.
# B_ calibrated (C=-480000) for ~2e-4 mean bias over <xn_i,xn_j> ~ N(0,1/64).
A_SCH = float(np.float32(4.0 * (2.0 ** 23) / math.log(2.0)))
B_SCH = float(np.float32(127 * 2 ** 23 - 4.0 * (2.0 ** 23) / math.log(2.0) - 480000.0))

# off-drain k -> use DVE bitcast-exp instead of ACT exp (Pool cannot touch
# PSUM or run fp32 ALU ops, so the exp wall splits across ACT/DVE only).
# With 2 PSUM work slots a third concurrent consumer stalls the rotation,
# so DVE only co-drains the post-gather tail.
def _dve_drain(k):
    return k >= 28 and (k % 2) == 1

_COLS = {"self": 0, "off": 0}  # filled at build; read by _finalize


def _emit_rsqrt(eng, pool, x_ap, out_ap, n, tag, order):
    """out = 1/sqrt(x) (bit-hack seed + 2 Newton steps) on engine `eng`."""
    MAGIC = 0x5F3759DF
    op = mybir.AluOpType
    ti = pool.tile([128, n], I32, tag=f"{tag}_ti", name=f"{tag}_ti")
    order(eng.tensor_scalar(
        out=ti[:], in0=x_ap.bitcast(I32), scalar1=1, scalar2=None,
        op0=op.logical_shift_right,
    ))
    yi = pool.tile([128, n], I32, tag=f"{tag}_yi", name=f"{tag}_yi")
    order(eng.tensor_scalar(
        out=yi[:], in0=ti[:], scalar1=-1, scalar2=None, op0=op.bitwise_xor
    ))
    order(eng.tensor_scalar(
        out=yi[:], in0=yi[:], scalar1=MAGIC + 1, scalar2=None, op0=op.add
    ))
    xh = pool.tile([128, n], F32, tag=f"{tag}_xh", name=f"{tag}_xh")
    order(eng.tensor_scalar(
        out=xh[:], in0=x_ap, scalar1=-0.5, scalar2=None, op0=op.mult
    ))
    cur = yi[:].bitcast(F32)
    for it in range(2):
        t2 = pool.tile([128, n], F32, tag=f"{tag}_t2", name=f"{tag}_t2")
        order(eng.tensor_tensor(out=t2[:], in0=cur, in1=cur, op=op.mult))
        order(eng.tensor_tensor(out=t2[:], in0=t2[:], in1=xh[:], op=op.mult))
        order(eng.tensor_scalar(
            out=t2[:], in0=t2[:], scalar1=1.5, scalar2=None, op0=op.add
        ))
        if it == 1:
            dst_ap = out_ap
        else:
            yt = pool.tile([128, n], F32, tag=f"{tag}_y", name=f"{tag}_y{it}")
            dst_ap = yt[:]
        order(eng.tensor_tensor(out=dst_ap, in0=cur, in1=t2[:], op=op.mult))
        cur = dst_ap


def _body(tc, wind, gidx, idn, acc):
    nc = tc.nc
    op = mybir.AluOpType
    DR = mybir.MatmulPerfMode.DoubleRow
    with (
        tc.tile_pool(name="persist", bufs=1) as P,
        tc.tile_pool(name="work", bufs=2) as W,
        tc.tile_pool(name="ps", bufs=2, space="PSUM") as PS,
    ):
        idx_sb = P.tile([128, NPAIR * 128], I16, tag="idx")

        accw = P.tile([128, ACC_W], F32, tag="accw")
        bias_o = P.tile([128, 1], F32, tag="bias_o")
        ident8 = P.tile([128, 128], F8, tag="ident8")

        # gathered rows, [128, band, DIM]: row i of slot c -> partition i%128,
        # band c*8 + i//128 (dma_gather's native layout)
        gath = P.tile([128, BANDS * DIM], F32, tag="gath")
        gath8 = P.tile([128, BANDS * DIM], F8, tag="gath8")
        # fp8 transposed layout: group g (4 bands = 512 rows) occupies cols
        # [g*1024, (g+1)*1024): [32 partitions, k-half h in {0,1}, 512 rows]
        xnT8 = P.tile([32, NGRP * 1024], F8, tag="xnT8")
        nsq = P.tile([128, BANDS], F32, tag="nsq")
        rinv = P.tile([128, BANDS], F32, tag="rinv")

        # queue-order pinning (the scheduler cost model mis-predicts gather
        # and PE readiness; pin each in-order engine to emission order)
        last = {"pe": None, "act": None, "dve": None, "pool": None}

        def _mk(key):
            def f(inst):
                if last[key] is not None:
                    add_dep_helper(inst.ins, last[key].ins, sync=False,
                                   reason=f"{key} order")
                last[key] = inst
                return inst
            return f

        pe_o, act_o, dve_o, pool_o = _mk("pe"), _mk("act"), _mk("dve"), _mk("pool")

        def gather(slot0, nslots):
            """one dma_gather for slots [slot0, slot0+nslots) out of window
            slot0//2 (idx data is window-relative)."""
            p = slot0 // 2
            n = nslots * CHUNK
            pool_o(nc.gpsimd.dma_gather(
                out_ap=gath[:, slot0 * 8 * DIM : (slot0 + nslots) * 8 * DIM]
                .rearrange("q (c d) -> q c d", d=DIM),
                in_ap=wind[p * WINDOW : (p + 1) * WINDOW, :],
                idxs_ap=idx_sb[:, slot0 * 64 : slot0 * 64 + n // 16],
                num_idxs=n,
                num_idxs_reg=n,
                elem_size=DIM,
            ))

        def setup_consts():
            warm = P.tile([128, 1], F32, tag="warm")
            act_o(nc.scalar.activation(
                out=warm[:], in_=bias_o[:],
                func=mybir.ActivationFunctionType.Exp,
            ))

        def norm_pool(c0, c1):
            """square + reduce + rsqrt on DVE for bands [c0, c1) (Pool's ALU
            is integer-only and has no PSUM/fp32 path)."""
            nb = c1 - c0
            g3 = gath[:, c0 * DIM : c1 * DIM].rearrange("p (c d) -> p c d", d=DIM)
            sq = W.tile([128, 16 * DIM], F32, tag="sq", name=f"sq{c0}")
            dve_o(nc.vector.tensor_tensor(
                out=sq[:, 0 : nb * DIM], in0=g3, in1=g3, op=op.mult))
            dve_o(nc.vector.tensor_reduce(
                out=nsq[:, c0:c1],
                in_=sq[:, 0 : nb * DIM].rearrange("p (c d) -> p c d", d=DIM),
                axis=mybir.AxisListType.X,
                op=op.add,
            ))
            _emit_rsqrt(nc.vector, W, nsq[:, c0:c1], rinv[:, c0:c1], nb,
                        f"rs{c0}", dve_o)

        def norm_mul(c0, c1):
            """fused normalize-multiply + fp8 cast on DVE."""
            nb = c1 - c0
            g3 = gath[:, c0 * DIM : c1 * DIM].rearrange("p (c d) -> p c d", d=DIM)
            r3 = (
                rinv[:, c0:c1]
                .rearrange("p (c o) -> p c o", o=1)
                .to_broadcast([128, nb, DIM])
            )
            g83 = gath8[:, c0 * DIM : c1 * DIM].rearrange("p (c d) -> p c d", d=DIM)
            dve_o(nc.vector.tensor_tensor(out=g83, in0=g3, in1=r3, op=op.mult))

        def transpose_group(g):
            """8 fp8 transposes (4 bands x 2 halves) -> [32,1024] PSUM, then
            DVE-copy the packed group into xnT8 (estep-2 PSUM constraint)."""
            pt8 = PS.tile([32, 2048], F8, tag="ps", name=f"tp{g}")
            for bi in range(4):
                c = g * 4 + bi
                for h in range(2):
                    s = 2 * (h * 512 + bi * 128)
                    pe_o(nc.tensor.transpose(
                        out=pt8[0:32, s : s + 256 : 2],
                        in_=gath8[:, c * DIM + h * 32 : c * DIM + (h + 1) * 32],
                        identity=ident8[:],
                    ))
            dve_o(nc.vector.tensor_copy(
                out=xnT8[:, g * 1024 : (g + 1) * 1024], in_=pt8[0:32, 0:2048:2]
            ))

        def rhs_ap(g, co, w):
            return xnT8[:, g * 1024 : (g + 1) * 1024].rearrange(
                "p (h c) -> p h c", h=2
            )[:, :, co : co + w]

        def lhs_ap(q, rt):
            return rhs_ap(q * 2 + rt // 4, (rt % 4) * 128, 128)

        # ---- rolling drain emitter: 2048-wide tiles, segments by kind ----
        st = {"tile": None, "fill": 0, "segs": [], "n": 0}
        cols = {"self": SELF_COL0, "off": OFF_COL0}
        offk = [0]
        dvek = [0]

        def mm_piece(q, rt, g, co, w, kind="off"):
            lhs = lhs_ap(q, rt)
            while w > 0:
                if st["tile"] is None:
                    st["tile"] = PS.tile(
                        [128, PSW], F32, tag="ps", name=f"mm{st['n']}"
                    )
                    st["n"] += 1
                take = min(w, PSW - st["fill"], 512 - st["fill"] % 512)
                pe_o(nc.tensor.matmul(
                    out=st["tile"][:, st["fill"] : st["fill"] + take],
                    lhsT=lhs,
                    rhs=rhs_ap(g, co, take),
                    start=True,
                    stop=True,
                    perf_mode=DR,
                ))
                if st["segs"] and st["segs"][-1][2] == kind \
                        and st["segs"][-1][1] == st["fill"]:
                    st["segs"][-1] = (st["segs"][-1][0], st["fill"] + take, kind)
                else:
                    st["segs"].append((st["fill"], st["fill"] + take, kind))
                st["fill"] += take
                co += take
                w -= take
                if st["fill"] == PSW:
                    flush()

        def drain_act(ap_in, col):
            act_o(nc.scalar.activation(
                out=ap_in,
                in_=ap_in,
                func=mybir.ActivationFunctionType.Exp,
                bias=bias_o[:],
                scale=4.0,
                accum_out=accw[:, col : col + 1],
            ))

        deferred_tr = []

        def drain_dve(ap_in, w, col):
            # convert PSUM->SBUF immediately (frees the PSUM slot for PE so
            # ACT never starves on the 2-slot rotation); the reduce of the
            # staged tile is deferred into DVE slack.
            k = dvek[0]
            dvek[0] += 1
            cv = P.tile([128, PSW], I32, tag=f"cv{k}", name=f"cv{k}")
            dve_o(nc.vector.tensor_scalar(
                out=cv[:, 0:w], in0=ap_in, scalar1=A_SCH, scalar2=B_SCH,
                op0=op.mult, op1=op.add,
            ))
            deferred_tr.append((cv, w, col))

        def emit_deferred_tr(n=None):
            todo = deferred_tr if n is None else deferred_tr[:n]
            for cv, w, col in todo:
                dve_o(nc.vector.tensor_reduce(
                    out=accw[:, col : col + 1],
                    in_=cv[:, 0:w].bitcast(F32),
                    axis=mybir.AxisListType.X,
                    op=op.add,
                ))
            del deferred_tr[: len(todo)]

        def flush():
            if st["fill"]:
                t = st["tile"]
                for (lo, hi, kind) in st["segs"]:
                    if kind == "self":
                        col = cols["self"]
                        cols["self"] += 1
                        drain_act(t[:, lo:hi], col)
                    else:
                        k = offk[0]
                        offk[0] += 1
                        col = cols["off"]
                        cols["off"] += 1
                        if _dve_drain(k):
                            drain_dve(t[:, lo:hi], hi - lo, col)
                        else:
                            drain_act(t[:, lo:hi], col)
            st["tile"] = None
            st["fill"] = 0
            st["segs"] = []
            # trickle one deferred reduce once it is a few tiles stale
            if len(deferred_tr) >= 3:
                emit_deferred_tr(1)

        def self_pass(q):
            for rt in range(8):
                mm_piece(q, rt, *(_lhs_loc(q, rt) + (128,)), kind="self")

        def _lhs_loc(q, rt):
            return (q * 2 + rt // 4, (rt % 4) * 128)

        def up_pass(q):
            # strict upper triangle of diag chunk q at 128-tile granularity
            for rt in range(8):
                s = (rt + 1) * 128
                for lo, hi in ((s, 512), (max(s, 512), 1024)):
                    if hi > lo:
                        mm_piece(q, rt, q * 2 + lo // 512, lo % 512, hi - lo)

        # ---- emission ----
        # NOTE: one dma_gather per 1024-row chunk. 2048-idx gathers emit 129
        # descriptors, one over the 128-deep SWDGE ring -> device lockup.
        # Gather descgen measures ~8.4ns/row of Pool-engine time (the cost
        # model's 0.34ns/desc is wrong for the gather ucode), so the ~52us
        # gather stream paces the kernel: the Pool queue carries NOTHING but
        # the six gathers (identity ships from host, memsets run on DVE) and
        # all downstream work is emitted in slot-availability order.
        nc.sync.dma_start(out=idx_sb[:], in_=gidx)
        nc.sync.dma_start(out=ident8[:], in_=idn)
        dve_o(nc.vector.memset(bias_o[:], -4.0))
        dve_o(nc.vector.memset(accw[:], 0.0))
        gather(0, 1)
        gather(1, 1)
        setup_consts()
        gather(2, 1)
        gather(3, 1)
        gather(4, 1)
        gather(5, 1)

        def norm_tp(slot):
            # drain any open partial tile first: the transpose PSUM allocs
            # rotate through the same pool slots, and waiting on an open
            # tile whose remaining fills are emitted later would deadlock
            flush()
            norm_pool(slot * 8, (slot + 1) * 8)
            norm_mul(slot * 8, (slot + 1) * 8)
            transpose_group(2 * slot)
            transpose_group(2 * slot + 1)

        # The transpose chain for slot s+1 is emitted BEFORE slot s's matmul
        # stream: slot-s matmuls are drain-paced (~12-18us), and with strict
        # PE queue order the next slot's transposes would otherwise sit
        # behind them, starving ACT for ~5us at every slot boundary.
        norm_tp(0)
        norm_tp(1)
        # slot 0: q0 self + upper
        self_pass(0)
        up_pass(0)
        norm_tp(2)
        # slot 1: sibling block + q1 self + upper
        for rt in range(8):
            mm_piece(0, rt, 2, 0, 512)
            mm_piece(0, rt, 3, 0, 512)
        self_pass(1)
        up_pass(1)
        norm_tp(3)
        # slot 2: q0/q1 x slot
        for q in (0, 1):
            for rt in range(8):
                mm_piece(q, rt, 4, 0, 512)
                mm_piece(q, rt, 5, 0, 512)
        norm_tp(4)
        # slot 3
        for q in (0, 1):
            for rt in range(8):
                mm_piece(q, rt, 6, 0, 512)
                mm_piece(q, rt, 7, 0, 512)
        norm_tp(5)
        # slot 4: q0 distance-4 half (half-swapped) + q1 full
        for rt in range(8):
            mm_piece(0, rt, 8 if rt < 4 else 9, 0, 512)
        for rt in range(8):
            mm_piece(1, rt, 8, 0, 512)
            mm_piece(1, rt, 9, 0, 512)
        # slot 5: q1 distance-4 half
        for rt in range(8):
            mm_piece(1, rt, 10 if rt < 4 else 11, 0, 512)
        flush()
        emit_deferred_tr()

        _COLS["self"] = cols["self"]
        _COLS["off"] = cols["off"]

        nc.sync.dma_start(out=acc, in_=accw[:])


def _build():
    nc = bacc.Bacc(
        "TRN2",
        target_bir_lowering=False,
        debug=False,
        enable_asserts=False,
        num_devices=NCORES,
    )
    wind = nc.dram_tensor("wind", [NPAIR * WINDOW, DIM], F32, kind="ExternalInput").ap()
    gidx = nc.dram_tensor("gidx", [128, NPAIR * 128], I16, kind="ExternalInput").ap()
    idn = nc.dram_tensor("idn", [128, 128], F8, kind="ExternalInput").ap()
    acc = nc.dram_tensor("acc", [128, ACC_W], F32, kind="ExternalOutput").ap()
    with tile.TileContext(nc) as tc:
        _body(tc, wind, gidx, idn, acc)
    nc.compile()
    return nc


_PROG = None


def _get_prog():
    global _PROG
    if _PROG is None:
        _PROG = _build()
    return _PROG


def _h(a):
    return 0 if a < 4 else 1


def _core_inputs(uid, pid, user_table, item_table, m):
    """per-core window tensor + int16 gather indices (sorted chunks)."""
    t = 0 if m < 4 else 1
    a1 = 2 * (m % 4)
    ids = [uid, pid][t]
    tab = [user_table, item_table][t]
    order = np.argsort(ids, kind="stable")
    svals = ids[order]

    slot_vals = []
    for i in range(NSLOT):
        c = (a1 + i) % 8
        v = svals[c * CHUNK : (c + 1) * CHUNK].copy()
        # distance-4 half-swap (complementary halves between core pairs)
        if i == 4 and _h(a1) == 1:
            v = np.concatenate([v[512:], v[:512]])
        if i == 5 and _h((a1 + 1) % 8) == 1:
            v = np.concatenate([v[512:], v[:512]])
        slot_vals.append(v)

    idx16 = np.zeros((128, NPAIR * 128), np.int16)
    bases = []
    for p in range(NPAIR):
        pairv = np.concatenate([slot_vals[2 * p], slot_vals[2 * p + 1]])
        base = min(int(pairv.min()), NROWS - WINDOW)
        assert int(pairv.max()) - base < WINDOW, (m, p)
        rel = (pairv - base).astype(np.int16)
        block = rel.reshape(128, 16).T  # idx i -> partition i%16, col i//16
        idx16[:, p * 128 : (p + 1) * 128] = np.tile(block, (8, 1))
        bases.append(base)

    wind = np.concatenate(
        [np.asarray(tab[b : b + WINDOW], dtype=np.float32) for b in bases], axis=0
    )
    import ml_dtypes
    idn = np.eye(128, dtype=np.float32).astype(ml_dtypes.float8_e4m3)
    return {"wind": np.ascontiguousarray(wind), "gidx": idx16, "idn": idn}


def _make_in_maps(user_id, pos_id, user_table, item_table):
    uid = np.asarray(user_id).astype(np.int64)
    pid = np.asarray(pos_id).astype(np.int64)
    ut = np.asarray(user_table, dtype=np.float32)
    it = np.asarray(item_table, dtype=np.float32)
    return [_core_inputs(uid, pid, ut, it, m) for m in range(NCORES)]


def _host_align(user_id, pos_id, user_table, item_table):
    ue = np.asarray(user_table, dtype=np.float64)[np.asarray(user_id)]
    pe = np.asarray(item_table, dtype=np.float64)[np.asarray(pos_id)]
    un = ue / np.linalg.norm(ue, axis=1, keepdims=True)
    pn = pe / np.linalg.norm(pe, axis=1, keepdims=True)
    return 2.0 - (2.0 / B) * float(np.einsum("ij,ij->", un, pn))


def _finalize(accs, align):
    """accs: list of [128, ACC_W] per core -> scalar loss."""
    _get_prog()
    a = np.stack([np.asarray(x, dtype=np.float64) for x in accs])
    ns, no = _COLS["self"], _COLS["off"]
    s_self_u = a[0:4, :, SELF_COL0:ns].sum()
    s_off_u = a[0:4, :, OFF_COL0:no].sum()
    s_self_p = a[4:8, :, SELF_COL0:ns].sum()
    s_off_p = a[4:8, :, OFF_COL0:no].sum()
    npairs = B * (B - 1) // 2
    pair_u = s_off_u + (s_self_u - B) / 2.0
    pair_p = s_off_p + (s_self_p - B) / 2.0
    unif = 0.5 * (np.log(pair_u / npairs) + np.log(pair_p / npairs))
    return np.asarray(align + unif, dtype=np.float32)


def _run(in_maps, trace=False, **kw):
    nc = _get_prog()
    return bass_utils.run_bass_kernel_spmd(
        nc, in_maps, core_ids=list(range(NCORES)), trace=trace, **kw
    )


def kernel(user_id, pos_id, neg_id=None, user_table=None, item_table=None):
    in_maps = _make_in_maps(user_id, pos_id, user_table, item_table)
    align = _host_align(user_id, pos_id, user_table, item_table)
    res = _run(in_maps, trace=False)
    return _finalize([res.results[m]["acc"] for m in range(NCORES)], align)


def _install_profile_hook():
    """The image's antenv lacks axon_hooks; shim it so trace=True can reach
    the NTFF profiler in libaxon_pjrt.so (same mechanism trn_boot uses)."""
    import sys
    import types

    if "antenv.axon_hooks" in sys.modules:
        return
    import antenv
    from trn_agent_boot.trn_boot import _ntff_profile_via_ctypes

    mod = types.ModuleType("antenv.axon_hooks")
    holder = [None]
    mod.set_axon_ntff_profile_hook = lambda h: holder.__setitem__(0, h)
    mod.get_axon_ntff_profile_hook = lambda: holder[0]
    sys.modules["antenv.axon_hooks"] = mod
    antenv.axon_hooks = mod
    mod.set_axon_ntff_profile_hook(
        _ntff_profile_via_ctypes("/opt/axon/libaxon_pjrt.so")
    )
    bass_utils.upload_artifacts = lambda tmpdir: ""


def run_profiled(user_id, pos_id, neg_id=None, user_table=None, item_table=None, **kw):
    _install_profile_hook()
    in_maps = _make_in_maps(user_id, pos_id, user_table, item_table)
    align = _host_align(user_id, pos_id, user_table, item_table)
    res = _run(in_maps, trace=True, **kw)
    out = _finalize([res.results[m]["acc"] for m in range(NCORES)], align)
    return out, res
